# revision 2
# baseline (speedup 1.0000x reference)
"""CARAFE upsampling (k=5, x2, C=256) v3 — Bass/Tile kernel on 8 NeuronCores.

v3 (ones-column denominator matmuls, software-pipelined emission) plus
2-pair units: both pairs of a unit scatter into pair-planes of one stage
buffer (SP/ACT queues in parallel) and come back in a single [120, 512]
readback, halving readback count and semaphore hops per pair; PE runs a
unit's 32 matmuls as one burst.

- Softmax denominators via an extra ones-column matmul: each (block, row)
  matmul pair reuses the stationary banded lhsT to also contract against
  an all-ones [120,1] rhs, dropping PSUM col 256 = sum(exp). This removes
  the whole second mask load + 32 exp+accum ops; the reciprocal is taken
  straight from PSUM. Denominator and numerator use the SAME bf16 exps,
  so softmax weights normalize exactly.
- Software-pipelined emission: ACT emits only exp ops, two pairs ahead;
  SP emits next pair's scatter before this pair's window load (whose WAR
  wait on the PE is then already satisfied); Pool does stage readbacks
  and output stores; DVE does reciprocals + PSUM eviction scaling.
- Stage and lhsT are triple-buffered so scatter[i+1]/readback[i+1]
  overlap matmul[i] without buffer-reuse stalls.
"""

import sys

for _p in ("/opt/trn_rl_repo",):
    if _p not in sys.path:
        sys.path.insert(0, _p)

import numpy as np

B = 2
H_IN = 64
W_IN = 64
C = 256
H_OUT = 128
W_OUT = 128
KK = 25
N_CORES = 8
ROWS_PER_CORE = H_OUT * B // N_CORES  # 32 output rows
PAIRS = ROWS_PER_CORE // 2  # 16
SLAB = PAIRS + 4  # feature rows a core touches
WPAD = W_IN + 4  # 68 padded feature columns
NBLK = 4
TW = 20
NSLOT = 6
KROWS = TW * NSLOT  # 120
NSTG = 6  # stage/lw pipeline depth (units)

_NC_CACHE = {}


def _build_nc(reps=1):
    import concourse.bacc as bacc
    import concourse.mybir as mybir
    from concourse import tile

    dt = mybir.dt
    f32 = dt.float32
    bf16 = dt.bfloat16
    Exp = mybir.ActivationFunctionType.Exp

    nc = bacc.Bacc("TRN2", target_bir_lowering=False, debug=False,
                   num_devices=N_CORES)
    feat = nc.dram_tensor("feat", [SLAB, WPAD, C], bf16, kind="ExternalInput")
    masks = nc.dram_tensor("masks", [ROWS_PER_CORE, W_OUT, KK], f32,
                           kind="ExternalInput")
    out = nc.dram_tensor("out", [ROWS_PER_CORE, W_OUT, C], bf16,
                         kind="ExternalOutput")

    AP = type(feat[:])

    with tile.TileContext(nc) as tc:
        with (
            tc.tile_pool(name="big", bufs=1) as big,
            tc.tile_pool(name="psum", bufs=8, space="PSUM") as psumpool,
        ):
            def mk(shape, tag, n, dtp):
                return [big.tile(shape, dtp, tag=f"{tag}{i}",
                                 name=f"{tag}{i}") for i in range(n)]

            lws = mk([KROWS, 4 * W_OUT], "lw", NSTG, bf16)
            m5all = mk([16, NBLK * ROWS_PER_CORE * 50], "m5all", 1, f32)[0]
            NE5 = NSLOT
            e5s = mk([16, 5 * NSLOT * 16], "e5", NE5, bf16)
            ivts = mk([128, 2], "ivt", 2, f32)
            ots = mk([128, 4 * C], "ot", 4, bf16)
            ones = mk([KROWS, 1], "ones", 1, bf16)[0]
            zt = mk([KROWS, 4 * W_OUT], "zt", 1, bf16)[0]
            # wnd2 last and single: both windows live in one tile (free
            # halves) so the sim shadow's span-boxing of the partition-
            # strided row loads stays inside the tile. Windows alternate
            # per 2-pair unit; a full-half DVE copy carries the 4
            # surviving ring rows across, so the row loads never WAR
            # against the immediately preceding matmuls.
            wnd2 = mk([KROWS, 2 * NBLK * C], "wnd", 1, bf16)[0]

            stage = nc.dram_tensor("lw_stage", [NSTG, 2, KROWS, 2 * W_OUT],
                                   bf16, kind="Internal")

            # ---- prologue ----
            # m5all first: the exp stream depends on it
            for b in range(NBLK):
                nc.sync.dma_start(
                    out=m5all[:, b * 1600:(b + 1) * 1600].rearrange(
                        "u (rr vp) -> u rr vp", rr=ROWS_PER_CORE),
                    in_=masks[:, 32 * b:32 * (b + 1)].rearrange(
                        "rr (u v) p -> u rr (v p)", v=2),
                )
            nc.vector.memset(wnd2[:], 0.0)  # also marks sim shadow
            nc.vector.memset(zt[:], 0.0)
            nc.vector.memset(ones[:], 1.0)
            for sb in range(NSTG):
                nc.sync.dma_start(
                    out=stage[sb].rearrange("ph k c -> k ph c"),
                    in_=zt[:].rearrange("k (ph c) -> k ph c", ph=2))

            # e5 dead-column pre-zero (buffer m serves pairs jj==m mod 6)
            for m in range(NE5):
                dead = (m % NSLOT + 5) % NSLOT
                v = e5s[m][:].rearrange("u (kx s w) -> u kx s w",
                                        kx=5, s=NSLOT)
                nc.vector.memset(v[:, :, dead], 0.0)

            WB = NBLK * C
            whalf = [wnd2[:, 0:WB], wnd2[:, WB:2 * WB]]
            wvs = [wnd2[:, wi * WB:(wi + 1) * WB].rearrange(
                "(t s) f -> t s f", s=NSLOT) for wi in range(2)]

            def load_wnd_row(wi, s, eng):
                if s >= SLAB:
                    return
                eng.dma_start(
                    out=wvs[wi][:, s % NSLOT, :].rearrange(
                        "t (b c) -> t b c", b=NBLK),
                    in_=AP(tensor=feat[:].tensor,
                           offset=feat[:].offset + s * WPAD * C,
                           ap=[[C, TW], [16 * C, NBLK], [1, C]]),
                )

            for s in range(NSLOT):
                load_wnd_row(0, s, nc.scalar)
            outq = []  # deferred output stores (emitted one unit late)

            e5vs = [e5[:].rearrange("u (kx s b v r) -> u b v s kx r",
                                    kx=5, s=NSLOT, b=NBLK, v=2)
                    for e5 in e5s]
            m5v = m5all[:].rearrange("u (b rr v ky kx) -> u b rr v ky kx",
                                     b=NBLK, rr=ROWS_PER_CORE, v=2, ky=5)
            lwvs = [lw[:].rearrange("k (ph b u v r) -> k ph b u v r",
                                    ph=2, b=NBLK, u=16, v=2) for lw in lws]

            def emit_exps(i):
                jj = i % PAIRS
                e5v = e5vs[jj % NE5]
                s0 = jj % NSLOT
                lenA = min(5, NSLOT - s0)
                runs = [(0, lenA, s0)]
                if lenA < 5:
                    runs.append((lenA, 5 - lenA, 0))
                for r in range(2):
                    for (ky0, ln, sA) in runs:
                        nc.scalar.activation(
                            out=e5v[:, :, :, sA:sA + ln, :, r],
                            in_=m5v[:, :, 2 * jj + r, :, ky0:ky0 + ln, :],
                            func=Exp,
                        )

            def emit_scatter(i, eng):
                jj = i % PAIRS
                e5 = e5s[jj % NE5]
                st = stage[(i // 2) % NSTG, i % 2]
                # stage[(u+kx)*6+s, 64b + 4u + 2v + r] <- e5[u, q*4 + w],
                # q = kx*24 + s*4 + b (dense), 4-el runs. The matmul needs
                # (u,v) adjacent in the column code (single free dim), so
                # 4-el runs are forced.
                eng.dma_start(
                    out=AP(tensor=st.tensor, offset=st.offset,
                           ap=[[NSLOT * 2 * W_OUT + 4, 16], [64, KROWS],
                               [1, 4]]),
                    in_=AP(tensor=e5[:].tensor, offset=e5[:].offset,
                           ap=[[5 * NSLOT * 16, 16], [4, KROWS], [1, 4]]),
                )

            def emit_readback(un):
                st = stage[un % NSTG]
                nc.gpsimd.dma_start(
                    out=lws[un % NSTG][:].rearrange(
                        "k (ph c) -> k ph c", ph=2),
                    in_=st[:].rearrange("ph k c -> k ph c"),
                )

            N = PAIRS * reps
            NU = N // 2
            for un in range(-3, NU):
                if un + 3 < NU:
                    emit_exps(2 * (un + 3))
                    emit_exps(2 * (un + 3) + 1)
                if 0 <= un + 2 < NU:
                    emit_scatter(2 * (un + 2), nc.sync)
                    emit_scatter(2 * (un + 2) + 1, nc.sync)
                if 0 <= un + 1 < NU:
                    emit_readback(un + 1)
                if un < 0:
                    continue
                # advance the ring into the other window for unit un+1:
                # copy all 6 slots (stale ones get overwritten by the row
                # loads below; their lhsT rows are zero for unit un+1
                # anyway), then load the two new rows.
                if un + 1 < NU:
                    nc.vector.tensor_copy(out=whalf[(un + 1) % 2],
                                          in_=whalf[un % 2])
                    load_wnd_row((un + 1) % 2,
                                 (2 * un) % PAIRS + NSLOT, nc.sync)
                    load_wnd_row((un + 1) % 2,
                                 (2 * un + 1) % PAIRS + NSLOT, nc.sync)
                wb0 = (un % 2) * WB
                lwv = lwvs[un % NSTG]
                ot = ots[un % 4]
                for ph in range(2):
                    i = 2 * un + ph
                    jj = i % PAIRS
                    ivt = ivts[i % 2]
                    for r in range(2):
                        ps = psumpool.tile([128, 2 * C], f32, tag="ps",
                                           name="ps")
                        for b in range(NBLK):
                            nc.tensor.matmul(ps[32 * b:32 * (b + 1), 0:C],
                                             lwv[:, ph, b, :, :, r],
                                             wnd2[:, wb0 + b * C:
                                                      wb0 + (b + 1) * C],
                                             start=True, stop=True,
                                             tile_position=(0, 32 * b))
                            nc.tensor.matmul(
                                ps[32 * b:32 * (b + 1), C:C + 1],
                                lwv[:, ph, b, :, :, r],
                                ones[:],
                                start=True, stop=True,
                                tile_position=(0, 32 * b))
                        nc.vector.reciprocal(ivt[:, r:r + 1],
                                             ps[:, C:C + 1])
                        nc.vector.tensor_scalar_mul(
                            ot[:, ph * 2 * C + r * C:
                               ph * 2 * C + (r + 1) * C],
                            ps[:, 0:C],
                            ivt[:, r:r + 1])
                outq.append(((2 * un + 1) % PAIRS, un))
                if len(outq) >= 3:
                    jo, uo = outq.pop(0)
                    nc.gpsimd.dma_start(
                        out=out[2 * jo - 2:2 * jo + 2].rearrange(
                            "q x c -> x q c"),
                        in_=ots[uo % 4][:].rearrange(
                            "x (q c) -> x q c", q=4),
                    )
            for jo, uo in outq:
                nc.gpsimd.dma_start(
                    out=out[2 * jo - 2:2 * jo + 2].rearrange(
                        "q x c -> x q c"),
                    in_=ots[uo % 4][:].rearrange(
                        "x (q c) -> x q c", q=4),
                )

    nc.compile()
    return nc


def get_nc(reps=1):
    key = reps
    if key not in _NC_CACHE:
        _NC_CACHE[key] = _build_nc(reps)
    return _NC_CACHE[key]


def shard_inputs(features, masks):
    import ml_dtypes
    features = np.asarray(features)
    masks = np.asarray(masks)
    in_maps = []
    for c in range(N_CORES):
        b, q = divmod(c, 4)
        y0 = PAIRS * q
        slab = np.zeros((SLAB, WPAD, C), np.float32)
        lo = y0 - 2
        for i in range(SLAB):
            y = lo + i
            if 0 <= y < H_IN:
                slab[i, 2:2 + W_IN] = features[b, y]
        in_maps.append({
            "feat": np.ascontiguousarray(slab).astype(ml_dtypes.bfloat16),
            "masks": np.ascontiguousarray(
                masks[b, ROWS_PER_CORE * q:ROWS_PER_CORE * (q + 1)]
            ).astype(np.float32),
        })
    return in_maps


def unshard_outputs(results):
    out = np.empty((B, H_OUT, W_OUT, C), np.float32)
    for c in range(N_CORES):
        b, q = divmod(c, 4)
        out[b, ROWS_PER_CORE * q:ROWS_PER_CORE * (q + 1)] = np.asarray(
            results[c]["out"]).astype(np.float32)
    return out


def kernel(features, masks):
    from concourse.bass_utils import run_bass_kernel_spmd

    nc = get_nc()
    in_maps = shard_inputs(features, masks)
    res = run_bass_kernel_spmd(nc, in_maps, list(range(N_CORES)))
    return unshard_outputs(res.results)
